# revision 13
# baseline (speedup 1.0000x reference)
"""nn_AGCB_Patch Bass/Tile kernel — data-parallel over batch across 8 NeuronCores.

Per spec sharding_hint: pure data parallelism over batch (B=8 -> 1 sample/core),
weights replicated. Each core runs one hand-written Bass/Tile program computing
the full AGCB block for its sample:
  - GCA channel gates: patchwise max-pool -> tiny non-local -> sigmoid
  - criss-cross attention on the 4 folded 64x64 patches (2 branches: identity
    and the +-45deg nearest-rotated branch; rotation maps are separable for
    exactly 45deg: ry=R(y-x), rx=R'(x+y), implemented as strided DVE copies)
  - 3x3 SAME conv + BN(eval) + gamma-residual + ReLU
Matmuls run in bf16 (tolerance 2e-2), accumulation f32 in PSUM.
"""

import os
import sys
import numpy as np

sys.path.insert(0, "/opt/trn_rl_repo")

_B, _C, _H, _W = 8, 256, 128, 128
HS = 64  # patch size
NPIX = HS * HS  # 4096


# ---------------------------------------------------------------- rotation maps
def _rot_scalars(angle_deg):
    th = np.deg2rad(np.float32(angle_deg)).astype(np.float32)
    c = np.cos(th).astype(np.float32)
    s = np.sin(th).astype(np.float32)
    return c, s


def _rotate_map(HH, WW, angle_deg):
    c, s = _rot_scalars(angle_deg)
    cy = np.float32((HH - 1) / 2.0)
    cx = np.float32((WW - 1) / 2.0)
    yy, xx = np.meshgrid(np.arange(HH, dtype=np.float32),
                         np.arange(WW, dtype=np.float32), indexing="ij")
    sx = c * (xx - cx) + s * (yy - cy) + cx
    sy = -s * (xx - cx) + c * (yy - cy) + cy
    rx, ry = np.round(sx), np.round(sy)
    valid = (rx >= 0) & (rx < WW) & (ry >= 0) & (ry < HH)
    ix = np.clip(rx, 0, WW - 1).astype(np.int32)
    iy = np.clip(ry, 0, HH - 1).astype(np.int32)
    return iy, ix, valid


def _sep_maps_p45():
    """Separable structure of the +45deg rotate (rotate-back).
    out[y,x] = src[R(u), Rv(v)] (if both in range else 0), u=y-x, v=x+y."""
    iy, ix, valid = _rotate_map(HS, HS, 45.0)
    Ru = {}
    Rv = {}
    for y in range(HS):
        for x in range(HS):
            u = y - x
            v = y + x
            if valid[y, x]:
                Ru[u] = int(iy[y, x])
                Rv[v] = int(ix[y, x])
    # runs of consecutive v where Rv steps by +1 (for step-A strided copies)
    runs = []
    vs = sorted(Rv)
    i = 0
    while i < len(vs):
        j = i
        while (j + 1 < len(vs) and vs[j + 1] == vs[j] + 1
               and Rv[vs[j + 1]] == Rv[vs[j]] + 1):
            j += 1
        runs.append((vs[i], Rv[vs[i]], j - i + 1))
        i = j + 1
    return Ru, Rv, runs, (iy, ix, valid)


# ---------------------------------------------------------------- bass program
_PROG_CACHE = {}


def _build_program(G1, G2, NLG):
    import concourse.bass as bass
    import concourse.mybir as mybir
    import concourse.tile as tile
    from concourse import bacc

    bf16 = mybir.dt.bfloat16
    f32 = mybir.dt.float32
    OP = mybir.AluOpType
    AF = mybir.ActivationFunctionType

    def fview(t, extra_off, dims):
        return bass.AP(tensor=t.tensor, offset=t.offset + extra_off,
                       ap=[list(t.ap[0])] + [list(d) for d in dims])

    def pview(t, p0, np_, extra_off, dims):
        ps = t.ap[0][0]
        return bass.AP(tensor=t.tensor, offset=t.offset + p0 * ps + extra_off,
                       ap=[[ps, np_]] + [list(d) for d in dims])

    def dview(t, extra_off, dims):
        return bass.AP(tensor=t.tensor, offset=t.offset + extra_off,
                       ap=[list(d) for d in dims])

    nc = bacc.Bacc(None, target_bir_lowering=False, debug=False)

    # ---- DRAM I/O
    xp_d = nc.declare_dram_parameter("xp", [8, 2, 128, NPIX], bf16, isOutput=False)
    xres_d = nc.declare_dram_parameter("xres", [2, 128, 16384], bf16, isOutput=False)
    wqkT_d = nc.declare_dram_parameter("wqkT", [2, 128, 64], bf16, isOutput=False)
    bqk_d = nc.declare_dram_parameter("bqk", [1, 64], bf16, isOutput=False)
    wvT_d = nc.declare_dram_parameter("wvT", [2, 128, 256], bf16, isOutput=False)
    bv_d = nc.declare_dram_parameter("bv", [1, 256], bf16, isOutput=False)
    wnqkT_d = nc.declare_dram_parameter("wnqkT", [2, 128, 16], bf16, isOutput=False)
    bnqk_d = nc.declare_dram_parameter("bnqk", [1, 16], bf16, isOutput=False)
    wnvT_d = nc.declare_dram_parameter("wnvT", [2, 128, 256], bf16, isOutput=False)
    bnv_d = nc.declare_dram_parameter("bnv", [1, 256], bf16, isOutput=False)
    kw_d = nc.declare_dram_parameter("kw", [2, 128, 2304], bf16, isOutput=False)
    ac_d = nc.declare_dram_parameter("acoef", [2, 128, 1], f32, isOutput=False)
    bc_d = nc.declare_dram_parameter("bcoef", [2, 128, 1], f32, isOutput=False)
    id_d = nc.declare_dram_parameter("ident", [128, 128], bf16, isOutput=False)
    dm_d = nc.declare_dram_parameter("diagmask", [128, 512], bf16, isOutput=False)
    sc_d = nc.declare_dram_parameter("scal", [1, 8], f32, isOutput=False)  # g1,g2,nlg
    out_d = nc.declare_dram_parameter("out", [2, 128, 16384], f32, isOutput=True)

    Ru, Rv, runsA, _ = _sep_maps_p45()

    with tile.TileContext(nc) as tc:
        import contextlib
        with contextlib.ExitStack() as ctx:
            cpool = ctx.enter_context(tc.tile_pool(name="consts", bufs=1))
            xpool = ctx.enter_context(tc.tile_pool(name="x", bufs=4))
            wpool = ctx.enter_context(tc.tile_pool(name="work", bufs=1))
            qkpool = ctx.enter_context(tc.tile_pool(name="qk", bufs=2))
            spool = ctx.enter_context(tc.tile_pool(name="small", bufs=2))
            opool = ctx.enter_context(tc.tile_pool(name="obuf", bufs=1))
            ccpool = ctx.enter_context(tc.tile_pool(name="convin", bufs=2))
            copool = ctx.enter_context(tc.tile_pool(name="convout", bufs=3))
            pspool = ctx.enter_context(tc.tile_pool(name="ps", bufs=5, space="PSUM"))
            trpool = ctx.enter_context(tc.tile_pool(name="pstr", bufs=2, space="PSUM"))
            dpool = ctx.enter_context(tc.tile_pool(name="dram", bufs=1, space="DRAM"))

            # ---- consts
            wqkT = [cpool.tile([128, 64], bf16, tag=f"wqkT{ct}", name=f"wqkT{ct}") for ct in range(2)]
            wvT = [cpool.tile([128, 256], bf16, tag=f"wvT{ct}", name=f"wvT{ct}") for ct in range(2)]
            wnqkT = [cpool.tile([128, 16], bf16, tag=f"wnqkT{ct}", name=f"wnqkT{ct}") for ct in range(2)]
            wnvT = [cpool.tile([128, 256], bf16, tag=f"wnvT{ct}", name=f"wnvT{ct}") for ct in range(2)]
            kw = [cpool.tile([128, 2304], bf16, tag=f"kw{ct}", name=f"kw{ct}") for ct in range(2)]
            acoef = [cpool.tile([128, 1], f32, tag=f"ac{ct}", name=f"ac{ct}") for ct in range(2)]
            bcoef = [cpool.tile([128, 1], f32, tag=f"bc{ct}", name=f"bc{ct}") for ct in range(2)]
            bqk = cpool.tile([1, 64], bf16, tag="bqk")
            bv = cpool.tile([1, 256], bf16, tag="bv")
            bnqk = cpool.tile([1, 16], bf16, tag="bnqk")
            bnv = cpool.tile([1, 256], bf16, tag="bnv")
            ident = cpool.tile([128, 128], bf16, tag="ident")
            diagmask = cpool.tile([128, 512], bf16, tag="diagmask")
            ones = cpool.tile([128, 128], bf16, tag="ones")
            ones_row = cpool.tile([1, 512], bf16, tag="ones_row")
            for ct in range(2):
                nc.sync.dma_start(out=wqkT[ct], in_=wqkT_d[ct])
                nc.sync.dma_start(out=wvT[ct], in_=wvT_d[ct])
                nc.sync.dma_start(out=wnqkT[ct], in_=wnqkT_d[ct])
                nc.sync.dma_start(out=wnvT[ct], in_=wnvT_d[ct])
                nc.sync.dma_start(out=kw[ct], in_=kw_d[ct])
                nc.sync.dma_start(out=acoef[ct], in_=ac_d[ct])
                nc.sync.dma_start(out=bcoef[ct], in_=bc_d[ct])
            nc.sync.dma_start(out=bqk, in_=bqk_d[:, :])
            nc.sync.dma_start(out=bv, in_=bv_d[:, :])
            nc.sync.dma_start(out=bnqk, in_=bnqk_d[:, :])
            nc.sync.dma_start(out=bnv, in_=bnv_d[:, :])
            nc.sync.dma_start(out=ident, in_=id_d[:, :])
            nc.sync.dma_start(out=diagmask, in_=dm_d[:, :])
            nc.vector.memset(ones, 1.0)
            nc.vector.memset(ones_row, 1.0)

            ctx_d = dpool.tile([2, 128, 16384], bf16)

            # ================= GCA gates (pool + tiny nonlocal + sigmoid)
            pooled = [cpool.tile([128, 4], bf16, tag=f"pooled{ct}", name=f"pooled{ct}") for ct in range(2)]
            for p in range(4):
                for ct in range(2):
                    xt = xpool.tile([128, NPIX], bf16, tag="x")
                    nc.sync.dma_start(out=xt, in_=xp_d[2 * p, ct])
                    nc.vector.tensor_reduce(out=pooled[ct][:, p:p + 1], in_=xt,
                                            axis=mybir.AxisListType.X, op=OP.max)
            # qn/kn: [8,4] each
            qn_ps = pspool.tile([8, 4], f32, tag="mm")
            kn_ps = pspool.tile([8, 4], f32, tag="mm")
            for ct in range(2):
                nc.tensor.matmul(qn_ps, wnqkT[ct][:, 0:8], pooled[ct],
                                 start=(ct == 0), stop=False)
            nc.tensor.matmul(qn_ps, bnqk[:, 0:8], ones_row[:, 0:4],
                             start=False, stop=True)
            for ct in range(2):
                nc.tensor.matmul(kn_ps, wnqkT[ct][:, 8:16], pooled[ct],
                                 start=(ct == 0), stop=False)
            nc.tensor.matmul(kn_ps, bnqk[:, 8:16], ones_row[:, 0:4],
                             start=False, stop=True)
            qn_sb = spool.tile([8, 4], bf16, tag="qn")
            kn_sb = spool.tile([8, 4], bf16, tag="kn")
            nc.vector.tensor_copy(qn_sb, qn_ps)
            nc.vector.tensor_copy(kn_sb, kn_ps)
            e4_ps = pspool.tile([4, 4], f32, tag="mm")
            nc.tensor.matmul(e4_ps, qn_sb, kn_sb, start=True, stop=True)
            e4_sb = spool.tile([4, 4], f32, tag="e4")
            nc.scalar.activation(e4_sb, e4_ps, AF.Exp)
            z4 = spool.tile([4, 1], f32, tag="z4")
            nc.vector.tensor_reduce(out=z4, in_=e4_sb, axis=mybir.AxisListType.X,
                                    op=OP.add)
            r4 = spool.tile([4, 1], f32, tag="r4")
            nc.vector.reciprocal(r4, z4)
            att4 = spool.tile([4, 4], bf16, tag="att4")
            nc.vector.tensor_scalar(out=att4, in0=e4_sb, scalar1=r4, scalar2=None,
                                    op0=OP.mult)
            at4_ps = pspool.tile([4, 4], bf16, tag="mm")
            nc.tensor.transpose(at4_ps, att4, ident[0:4, 0:4])
            attT4 = spool.tile([4, 4], bf16, tag="attT4")
            nc.vector.tensor_copy(attT4, at4_ps)
            vnT_ps = pspool.tile([4, 256], f32, tag="mm")
            for ct in range(2):
                nc.tensor.matmul(vnT_ps, pooled[ct], wnvT[ct],
                                 start=(ct == 0), stop=False)
            nc.tensor.matmul(vnT_ps, ones_row[:, 0:4], bnv, start=False, stop=True)
            vnT_sb = spool.tile([4, 256], bf16, tag="vnT")
            nc.vector.tensor_copy(vnT_sb, vnT_ps)
            gca = [cpool.tile([128, 4], f32, tag=f"gca{ct}", name=f"gca{ct}") for ct in range(2)]
            for ct in range(2):
                g_ps = pspool.tile([128, 4], f32, tag="mm")
                nc.tensor.matmul(g_ps, vnT_sb[:, ct * 128:(ct + 1) * 128], attT4,
                                 start=True, stop=True)
                gt = spool.tile([128, 4], f32, tag="gt")
                nc.vector.scalar_tensor_tensor(out=gt, in0=g_ps, scalar=float(NLG),
                                               in1=pooled[ct], op0=OP.mult,
                                               op1=OP.add)
                nc.scalar.activation(gca[ct], gt, AF.Sigmoid)

            # ================= main loop: 8 images (4 patches x 2 branches)
            O_b0 = [opool.tile([128, NPIX], bf16, tag=f"o0_{ct}", name=f"o0_{ct}") for ct in range(2)]
            O_b1 = [opool.tile([128, NPIX], bf16, tag=f"o1_{ct}", name=f"o1_{ct}") for ct in range(2)]
            x_b0 = [None, None]

            for img in range(8):
                p, br = img // 2, img % 2
                O = O_b0 if br == 0 else O_b1

                xt = [xpool.tile([128, NPIX], bf16, tag="x", name="xt") for _ in range(2)]
                for ct in range(2):
                    nc.sync.dma_start(out=xt[ct], in_=xp_d[img, ct])
                if br == 0:
                    x_b0 = xt

                # ---- projections q,k (psum [64, 512] chunks)
                q_sb = qkpool.tile([32, NPIX], bf16, tag="qk")
                k_sb = qkpool.tile([32, NPIX], bf16, tag="qk")
                for chv in range(8):
                    sl = slice(chv * 512, (chv + 1) * 512)
                    ps = pspool.tile([64, 512], f32, tag="mm")
                    for ct in range(2):
                        nc.tensor.matmul(ps, wqkT[ct], xt[ct][:, sl],
                                         start=(ct == 0), stop=False)
                    nc.tensor.matmul(ps, bqk, ones_row, start=False, stop=True)
                    nc.vector.tensor_copy(q_sb[:, sl], ps[0:32, :])
                    nc.vector.tensor_copy(k_sb[:, sl], ps[32:64, :])
                # ---- projection v
                v_sb = [wpool.tile([128, NPIX], bf16, tag=f"v_{ct}", name=f"v_{ct}") for ct in range(2)]
                for co in range(2):
                    for chv in range(8):
                        sl = slice(chv * 512, (chv + 1) * 512)
                        ps = pspool.tile([128, 512], f32, tag="mm")
                        for ci in range(2):
                            nc.tensor.matmul(ps, wvT[ci][:, co * 128:(co + 1) * 128],
                                             xt[ci][:, sl], start=(ci == 0), stop=False)
                        nc.tensor.matmul(ps, bv[:, co * 128:(co + 1) * 128],
                                         ones_row, start=False, stop=True)
                        nc.scalar.activation(v_sb[co][:, sl], ps, AF.Copy)

                # ---- v transposes: vT2 (column-major slices), all base partition 0
                vT2 = [wpool.tile([64, 8192], bf16, tag=f"vt2_{ct}", name=f"vt2_{ct}")
                       for ct in range(2)]
                for ct in range(2):
                    for qb in range(16):  # 4 single-col transposes per psum tile
                        tps = trpool.tile([64, 512], bf16, tag="tr", name="tps")
                        for k4 in range(4):
                            w = qb * 4 + k4
                            in1 = fview(v_sb[ct], w, [[64, 64]])
                            nc.tensor.matmul(tps[:, k4 * 128:(k4 + 1) * 128],
                                             in1, ident, start=True, stop=True,
                                             is_transpose=True)
                        nc.scalar.activation(vT2[ct][:, qb * 512:(qb + 1) * 512],
                                             tps, AF.Copy)

                # ---- eH scores + softmax numerators + ZH (+ oH below)
                attH = wpool.tile([64, 4096], bf16, tag="attH")
                attW = wpool.tile([64, 4096], bf16, tag="attW")
                Zt = wpool.tile([128, NPIX], bf16, tag="Zt")
                for grp in range(8):
                    eps = pspool.tile([64, 512], f32, tag="mm", name="eps")
                    for wl in range(8):
                        w = grp * 8 + wl
                        lhsT = fview(k_sb, w, [[64, 64]])
                        rhs = fview(q_sb, w, [[64, 64]])
                        nc.tensor.matmul(eps[:, wl * 64:(wl + 1) * 64], lhsT, rhs,
                                         start=True, stop=True)
                    asl = attH[:, grp * 512:(grp + 1) * 512]
                    nc.scalar.activation(asl, eps, AF.Exp)
                    nc.vector.tensor_mul(asl, asl, diagmask[0:64, :])
                    zH = pspool.tile([128, 512], f32, tag="mm", name="zH")
                    nc.tensor.matmul(zH, ones[0:64, :], asl, start=True, stop=True)
                    nc.vector.tensor_copy(
                        fview(Zt, grp * 8, [[1, 8], [64, 64]]),
                        fview(zH, 0, [[64, 8], [1, 64]]))
                # ---- oH (uses vT2)
                for grp8 in range(8):
                    for ct in range(2):
                        ops_ = pspool.tile([128, 512], f32, tag="mm", name="ops")
                        for wl in range(8):
                            w = grp8 * 8 + wl
                            lhsT = fview(vT2[ct], w * 128, [[1, 128]])
                            rhs = fview(attH, w * 64, [[1, 64]])
                            nc.tensor.matmul(ops_[:, wl * 64:(wl + 1) * 64],
                                             lhsT, rhs, start=True, stop=True)
                        nc.vector.tensor_copy(
                            fview(O[ct], grp8 * 8, [[1, 8], [64, 64]]),
                            fview(ops_, 0, [[64, 8], [1, 64]]))

                # ---- vT3 (row slices) into the SAME buffers as vT2 (oH done)
                vT3 = [wpool.tile([64, 8192], bf16, tag=f"vt2_{ct}", name=f"vt3_{ct}")
                       for ct in range(2)]
                for ct in range(2):
                    for qb in range(16):
                        tps = trpool.tile([64, 512], bf16, tag="tr", name="tps3")
                        for k4 in range(4):
                            h = qb * 4 + k4
                            in1 = fview(v_sb[ct], h * 64, [[1, 64]])
                            nc.tensor.matmul(tps[:, k4 * 128:(k4 + 1) * 128],
                                             in1, ident, start=True, stop=True,
                                             is_transpose=True)
                        nc.scalar.activation(vT3[ct][:, qb * 512:(qb + 1) * 512],
                                             tps, AF.Copy)
                # ---- eW + ZW + oW
                for grp in range(8):
                    fps = pspool.tile([64, 512], f32, tag="mm", name="fps")
                    for hl in range(8):
                        h = grp * 8 + hl
                        lhsT = fview(k_sb, h * 64, [[1, 64]])
                        rhs = fview(q_sb, h * 64, [[1, 64]])
                        nc.tensor.matmul(fps[:, hl * 64:(hl + 1) * 64], lhsT, rhs,
                                         start=True, stop=True)
                    asl = attW[:, grp * 512:(grp + 1) * 512]
                    nc.scalar.activation(asl, fps, AF.Exp)
                    zW = pspool.tile([128, 512], f32, tag="mm", name="zW")
                    nc.tensor.matmul(zW, ones[0:64, :], asl, start=True, stop=True)
                    sl_e = fview(Zt, (grp * 8) * 64, [[64, 8], [1, 64]])
                    nc.vector.tensor_tensor(out=sl_e, in0=sl_e,
                                            in1=fview(zW, 0, [[64, 8], [1, 64]]),
                                            op=OP.add)
                for grp8 in range(8):
                    for ct in range(2):
                        ops_ = pspool.tile([128, 512], f32, tag="mm", name="opsw")
                        for hl in range(8):
                            h = grp8 * 8 + hl
                            lhsT = fview(vT3[ct], h * 128, [[1, 128]])
                            rhs = fview(attW, h * 64, [[1, 64]])
                            nc.tensor.matmul(ops_[:, hl * 64:(hl + 1) * 64],
                                             lhsT, rhs, start=True, stop=True)
                        osl = fview(O[ct], grp8 * 8 * 64, [[64, 8], [1, 64]])
                        nc.vector.tensor_tensor(
                            out=osl, in0=osl,
                            in1=fview(ops_, 0, [[64, 8], [1, 64]]), op=OP.add)

                # ---- O *= gamma * R  (R = 1/Zt), chunked f32 reciprocal
                gsc = float(G1) if br == 0 else float(G2)
                for chv in range(4):
                    sl = slice(chv * 1024, (chv + 1) * 1024)
                    rch = spool.tile([128, 1024], f32, tag="rch")
                    nc.vector.reciprocal(rch, Zt[:, sl])
                    for ct in range(2):
                        nc.vector.scalar_tensor_tensor(
                            out=O[ct][:, sl], in0=O[ct][:, sl], scalar=gsc,
                            in1=rch, op0=OP.mult, op1=OP.mult)

                if br == 1:
                    # ---- rotate O_b1 back by +45 into rotG; combine; write ctx
                    for ct in range(2):
                        tmpA = wpool.tile([128, 8192], bf16, tag="vt2_0")
                        nc.gpsimd.memset(tmpA, 0.0)
                        for (v0, c0, L) in runsA:
                            nc.vector.tensor_copy(
                                fview(tmpA, v0, [[128, 64], [1, L]]),
                                fview(O_b1[ct], c0, [[64, 64], [1, L]]))
                        rotG = wpool.tile([128, NPIX], bf16, tag="v_1")
                        nc.gpsimd.memset(rotG, 0.0)
                        for u in range(-63, 64):
                            if u not in Ru:
                                continue
                            y0 = max(0, u)
                            y1 = min(63, 63 + u)
                            L = y1 - y0 + 1
                            nc.vector.tensor_copy(
                                fview(rotG, 65 * y0 - u, [[65, L]]),
                                fview(tmpA, Ru[u] * 128 + 2 * y0 - u, [[2, L]]))
                        # ctx = (O_b0 + rotG + x) * gate
                        nc.vector.tensor_tensor(out=O_b0[ct], in0=O_b0[ct],
                                                in1=rotG, op=OP.add)
                        nc.vector.tensor_tensor(out=O_b0[ct], in0=O_b0[ct],
                                                in1=x_b0[ct], op=OP.add)
                        ctxt = wpool.tile([128, NPIX], bf16, tag="v_0")
                        nc.vector.tensor_scalar(out=ctxt, in0=O_b0[ct],
                                                scalar1=gca[ct][:, p:p + 1],
                                                scalar2=None, op0=OP.mult)
                        pr, pc = p // 2, p % 2
                        dst = dview(ctx_d, ct * (128 * 16384) + (pr * 64) * 128
                                    + pc * 64,
                                    [[16384, 128], [128, 64], [1, 64]])
                        nc.sync.dma_start(out=dst, in_=ctxt)

            # ================= conv 3x3 + BN + residual ReLU
            for chk in range(32):
                r0 = chk * 4
                vs = max(0, r0 - 1)
                ve = min(127, r0 + 4)
                nrows = ve - vs + 1
                j0 = vs - (r0 - 1)
                cc = []
                for ct in range(2):
                    t = ccpool.tile([128, 780], bf16, tag=f"cc{ct}")
                    nc.gpsimd.memset(t, 0.0)
                    src = dview(ctx_d, ct * (128 * 16384) + vs * 128,
                                [[16384, 128], [128, nrows], [1, 128]])
                    nc.sync.dma_start(
                        out=fview(t, j0 * 130 + 1, [[130, nrows], [1, 128]]),
                        in_=src)
                    cc.append(t)
                for ko in range(2):
                    ps = pspool.tile([128, 512], f32, tag="mm")
                    first = True
                    for tap in range(9):
                        dy, dx = tap // 3, tap % 3
                        for ki in range(2):
                            last = (tap == 8 and ki == 1)
                            nc.tensor.matmul(
                                ps, kw[ki][:, (tap * 2 + ko) * 128:
                                           (tap * 2 + ko + 1) * 128],
                                fview(cc[ki], dy * 130 + dx, [[130, 4], [1, 128]]),
                                start=first, stop=last)
                            first = False
                    xrt = copool.tile([128, 512], bf16, tag="xr")
                    nc.sync.dma_start(out=xrt,
                                      in_=xres_d[ko, :, r0 * 128:(r0 + 4) * 128])
                    t1 = copool.tile([128, 512], bf16, tag="t1")
                    nc.vector.tensor_scalar(out=t1, in0=ps, scalar1=acoef[ko],
                                            scalar2=bcoef[ko], op0=OP.mult,
                                            op1=OP.add)
                    t2 = copool.tile([128, 512], bf16, tag="t2")
                    nc.vector.tensor_tensor(out=t2, in0=t1, in1=xrt, op=OP.add)
                    osb = copool.tile([128, 512], f32, tag="osb")
                    nc.scalar.activation(osb, t2, AF.Relu)
                    nc.sync.dma_start(out=out_d[ko, :, r0 * 128:(r0 + 4) * 128],
                                      in_=osb)

    nc.finalize()
    return nc


def _get_program(g1, g2, nlg):
    key = (round(float(g1), 10), round(float(g2), 10), round(float(nlg), 10))
    if key not in _PROG_CACHE:
        _PROG_CACHE[key] = _build_program(g1, g2, nlg)
    return _PROG_CACHE[key]


# ---------------------------------------------------------------- host glue
def _prep_inputs(x, wq, bq, wk, bk, wv, bv, nq_w, nq_b, nk_w, nk_b, nv_w, nv_b,
                 conv_w, conv_b, bn_w, bn_b, bn_mean, bn_var, gamma):
    import ml_dtypes
    bf = ml_dtypes.bfloat16
    B = x.shape[0]
    f32 = np.float32

    iy, ix, valid = _rotate_map(HS, HS, -45.0)
    gmap = (iy * HS + ix).reshape(-1)
    vmask = valid.reshape(-1).astype(f32)

    # patches: xp[img, ct, c_local, pix]; img = 2*p + b, p = pr*2+pc
    x5 = x.reshape(B, 256, 2, HS, 2, HS).transpose(0, 2, 4, 1, 3, 5)
    # x5: (B, pr, pc, C, h, w)
    xp_all = []
    for n in range(B):
        per = np.empty((8, 2, 128, NPIX), dtype=bf)
        for p in range(4):
            pr, pc = p // 2, p % 2
            img = x5[n, pr, pc].reshape(256, NPIX).astype(f32)
            imgr = img[:, gmap] * vmask[None, :]
            per[2 * p + 0] = img.reshape(2, 128, NPIX).astype(bf)
            per[2 * p + 1] = imgr.reshape(2, 128, NPIX).astype(bf)
        xp_all.append(per)

    xres_all = [x[n].reshape(2, 128, 16384).astype(bf) for n in range(B)]

    wqkT = np.stack([np.concatenate([wq.T, wk.T], axis=1)[ct * 128:(ct + 1) * 128]
                     for ct in range(2)]).astype(bf)  # (2,128,64)
    bqk = np.concatenate([bq, bk])[None, :].astype(bf)
    wvT = np.stack([wv.T[ct * 128:(ct + 1) * 128] for ct in range(2)]).astype(bf)
    bvr = bv[None, :].astype(bf)
    wnqkT = np.stack([np.concatenate([nq_w.T, nk_w.T], axis=1)
                      [ct * 128:(ct + 1) * 128] for ct in range(2)]).astype(bf)
    bnqk = np.concatenate([nq_b, nk_b])[None, :].astype(bf)
    wnvT = np.stack([nv_w.T[ct * 128:(ct + 1) * 128] for ct in range(2)]).astype(bf)
    bnv = nv_b[None, :].astype(bf)

    kw = np.empty((2, 128, 2304), dtype=bf)
    for ki in range(2):
        for tap in range(9):
            dy, dx = tap // 3, tap % 3
            for ko in range(2):
                blk = conv_w[ko * 128:(ko + 1) * 128,
                             ki * 128:(ki + 1) * 128, dy, dx].T  # (128 in,128 out)
                kw[ki, :, (tap * 2 + ko) * 128:(tap * 2 + ko + 1) * 128] = \
                    blk.astype(bf)

    rs = 1.0 / np.sqrt(bn_var + 1e-5)
    A = (gamma * bn_w * rs).astype(f32)
    Bc = (gamma * ((conv_b - bn_mean) * rs * bn_w + bn_b)).astype(f32)
    ac = A.reshape(2, 128, 1)
    bc = Bc.reshape(2, 128, 1)

    ident = np.eye(128, dtype=f32).astype(bf)
    dm = np.ones((128, 512), dtype=f32)
    for pp in range(128):
        g = pp % 64
        for blk in range(8):
            dm[pp, blk * 64 + g] = 0.0
    dm = dm.astype(bf)

    consts = dict(wqkT=wqkT, bqk=bqk, wvT=wvT, bv=bvr, wnqkT=wnqkT, bnqk=bnqk,
                  wnvT=wnvT, bnv=bnv, kw=kw, acoef=ac, bcoef=bc, ident=ident,
                  diagmask=dm, scal=np.zeros((1, 8), np.float32))
    return xp_all, xres_all, consts


def kernel(**inputs) -> np.ndarray:
    x = np.asarray(inputs["x"], np.float32)
    assert int(inputs["scale"]) == 2
    g1 = float(np.asarray(inputs["gamma1"]))
    g2 = float(np.asarray(inputs["gamma2"]))
    nlg = float(np.asarray(inputs["nl_gamma"]))
    gamma = float(np.asarray(inputs["gamma"]))

    xp_all, xres_all, consts = _prep_inputs(
        x, np.asarray(inputs["wq"], np.float32), np.asarray(inputs["bq"], np.float32),
        np.asarray(inputs["wk"], np.float32), np.asarray(inputs["bk"], np.float32),
        np.asarray(inputs["wv"], np.float32), np.asarray(inputs["bv"], np.float32),
        np.asarray(inputs["nq_w"], np.float32), np.asarray(inputs["nq_b"], np.float32),
        np.asarray(inputs["nk_w"], np.float32), np.asarray(inputs["nk_b"], np.float32),
        np.asarray(inputs["nv_w"], np.float32), np.asarray(inputs["nv_b"], np.float32),
        np.asarray(inputs["conv_w"], np.float32), np.asarray(inputs["conv_b"], np.float32),
        np.asarray(inputs["bn_w"], np.float32), np.asarray(inputs["bn_b"], np.float32),
        np.asarray(inputs["bn_mean"], np.float32), np.asarray(inputs["bn_var"], np.float32),
        gamma)

    nc = _get_program(g1, g2, nlg)

    in_maps = []
    for c in range(8):
        m = dict(consts)
        m["xp"] = xp_all[c]
        m["xres"] = xres_all[c]
        in_maps.append(m)

    if os.environ.get("AGCB_SIM", "0") == "1":
        from concourse.bass_interp import CoreSim
        sim = CoreSim(nc)
        for name, arr in in_maps[0].items():
            sim.tensor(name)[:] = arr
        sim.simulate()
        outs = [np.asarray(sim.tensor("out"), np.float32)]
        res = np.repeat(outs[0][None], 8, axis=0)
    else:
        from concourse.bass_utils import run_bass_kernel_spmd
        trace = os.environ.get("AGCB_TRACE", "0") == "1"
        r = run_bass_kernel_spmd(nc, in_maps, list(range(8)), trace=trace)
        global _LAST_EXEC_NS
        _LAST_EXEC_NS = r.exec_time_ns
        if trace and r.instructions_and_trace is not None:
            print(f"[kernel] trace: {r.instructions_and_trace[1]}", file=sys.stderr)
        res = np.stack([rm["out"] for rm in r.results])

    out = res.reshape(8, 256, 128, 128).astype(np.float32)
    return out


_LAST_EXEC_NS = None


# revision 14
# speedup vs baseline: 1.2083x; 1.2083x over previous
"""nn_AGCB_Patch Bass/Tile kernel — data-parallel over batch across 8 NeuronCores.

Per spec sharding_hint: pure data parallelism over batch (B=8 -> 1 sample/core),
weights replicated. Each core runs one hand-written Bass/Tile program computing
the full AGCB block for its sample:
  - GCA channel gates: patchwise max-pool -> tiny non-local -> sigmoid
  - criss-cross attention on the 4 folded 64x64 patches (2 branches: identity
    and the +-45deg nearest-rotated branch; rotation maps are separable for
    exactly 45deg: ry=R(y-x), rx=R'(x+y), implemented as strided DVE copies)
  - 3x3 SAME conv + BN(eval) + gamma-residual + ReLU
Matmuls run in bf16 (tolerance 2e-2), accumulation f32 in PSUM.
"""

import os
import sys
import numpy as np

sys.path.insert(0, "/opt/trn_rl_repo")

_B, _C, _H, _W = 8, 256, 128, 128
HS = 64  # patch size
NPIX = HS * HS  # 4096


# ---------------------------------------------------------------- rotation maps
def _rot_scalars(angle_deg):
    th = np.deg2rad(np.float32(angle_deg)).astype(np.float32)
    c = np.cos(th).astype(np.float32)
    s = np.sin(th).astype(np.float32)
    return c, s


def _rotate_map(HH, WW, angle_deg):
    c, s = _rot_scalars(angle_deg)
    cy = np.float32((HH - 1) / 2.0)
    cx = np.float32((WW - 1) / 2.0)
    yy, xx = np.meshgrid(np.arange(HH, dtype=np.float32),
                         np.arange(WW, dtype=np.float32), indexing="ij")
    sx = c * (xx - cx) + s * (yy - cy) + cx
    sy = -s * (xx - cx) + c * (yy - cy) + cy
    rx, ry = np.round(sx), np.round(sy)
    valid = (rx >= 0) & (rx < WW) & (ry >= 0) & (ry < HH)
    ix = np.clip(rx, 0, WW - 1).astype(np.int32)
    iy = np.clip(ry, 0, HH - 1).astype(np.int32)
    return iy, ix, valid


def _sep_maps_p45():
    """Separable structure of the +45deg rotate (rotate-back).
    out[y,x] = src[R(u), Rv(v)] (if both in range else 0), u=y-x, v=x+y."""
    iy, ix, valid = _rotate_map(HS, HS, 45.0)
    Ru = {}
    Rv = {}
    for y in range(HS):
        for x in range(HS):
            u = y - x
            v = y + x
            if valid[y, x]:
                Ru[u] = int(iy[y, x])
                Rv[v] = int(ix[y, x])
    # runs of consecutive v where Rv steps by +1 (for step-A strided copies)
    runs = []
    vs = sorted(Rv)
    i = 0
    while i < len(vs):
        j = i
        while (j + 1 < len(vs) and vs[j + 1] == vs[j] + 1
               and Rv[vs[j + 1]] == Rv[vs[j]] + 1):
            j += 1
        runs.append((vs[i], Rv[vs[i]], j - i + 1))
        i = j + 1
    return Ru, Rv, runs, (iy, ix, valid)


# ---------------------------------------------------------------- bass program
_PROG_CACHE = {}


def _build_program(G1, G2, NLG):
    import concourse.bass as bass
    import concourse.mybir as mybir
    import concourse.tile as tile
    from concourse import bacc

    bf16 = mybir.dt.bfloat16
    f32 = mybir.dt.float32
    OP = mybir.AluOpType
    AF = mybir.ActivationFunctionType

    def fview(t, extra_off, dims):
        return bass.AP(tensor=t.tensor, offset=t.offset + extra_off,
                       ap=[list(t.ap[0])] + [list(d) for d in dims])

    def pview(t, p0, np_, extra_off, dims):
        ps = t.ap[0][0]
        return bass.AP(tensor=t.tensor, offset=t.offset + p0 * ps + extra_off,
                       ap=[[ps, np_]] + [list(d) for d in dims])

    def dview(t, extra_off, dims):
        return bass.AP(tensor=t.tensor, offset=t.offset + extra_off,
                       ap=[list(d) for d in dims])

    nc = bacc.Bacc(None, target_bir_lowering=False, debug=False)

    # ---- DRAM I/O
    xp_d = nc.declare_dram_parameter("xp", [8, 2, 128, NPIX], bf16, isOutput=False)
    xres_d = nc.declare_dram_parameter("xres", [2, 128, 16384], bf16, isOutput=False)
    wqkT_d = nc.declare_dram_parameter("wqkT", [2, 128, 64], bf16, isOutput=False)
    bqk_d = nc.declare_dram_parameter("bqk", [1, 64], bf16, isOutput=False)
    wvT_d = nc.declare_dram_parameter("wvT", [2, 128, 256], bf16, isOutput=False)
    bv_d = nc.declare_dram_parameter("bv", [1, 256], bf16, isOutput=False)
    wnqkT_d = nc.declare_dram_parameter("wnqkT", [2, 128, 16], bf16, isOutput=False)
    bnqk_d = nc.declare_dram_parameter("bnqk", [1, 16], bf16, isOutput=False)
    wnvT_d = nc.declare_dram_parameter("wnvT", [2, 128, 256], bf16, isOutput=False)
    bnv_d = nc.declare_dram_parameter("bnv", [1, 256], bf16, isOutput=False)
    kw_d = nc.declare_dram_parameter("kw", [2, 128, 2304], bf16, isOutput=False)
    ac_d = nc.declare_dram_parameter("acoef", [2, 128, 1], f32, isOutput=False)
    bc_d = nc.declare_dram_parameter("bcoef", [2, 128, 1], f32, isOutput=False)
    id_d = nc.declare_dram_parameter("ident", [128, 128], bf16, isOutput=False)
    dm_d = nc.declare_dram_parameter("diagmask", [128, 512], bf16, isOutput=False)
    sc_d = nc.declare_dram_parameter("scal", [1, 8], f32, isOutput=False)  # g1,g2,nlg
    out_d = nc.declare_dram_parameter("out", [2, 128, 16384], bf16, isOutput=True)

    Ru, Rv, runsA, _ = _sep_maps_p45()

    with tile.TileContext(nc) as tc:
        import contextlib
        with contextlib.ExitStack() as ctx:
            cpool = ctx.enter_context(tc.tile_pool(name="consts", bufs=1))
            xpool = ctx.enter_context(tc.tile_pool(name="x", bufs=4))
            wpool = ctx.enter_context(tc.tile_pool(name="work", bufs=1))
            qkpool = ctx.enter_context(tc.tile_pool(name="qk", bufs=2))
            spool = ctx.enter_context(tc.tile_pool(name="small", bufs=2))
            opool = ctx.enter_context(tc.tile_pool(name="obuf", bufs=1))
            ccpool = ctx.enter_context(tc.tile_pool(name="convin", bufs=2))
            copool = ctx.enter_context(tc.tile_pool(name="convout", bufs=3))
            pspool = ctx.enter_context(tc.tile_pool(name="ps", bufs=5, space="PSUM"))
            trpool = ctx.enter_context(tc.tile_pool(name="pstr", bufs=2, space="PSUM"))
            dpool = ctx.enter_context(tc.tile_pool(name="dram", bufs=1, space="DRAM"))

            # ---- consts
            wqkT = [cpool.tile([128, 64], bf16, tag=f"wqkT{ct}", name=f"wqkT{ct}") for ct in range(2)]
            wvT = [cpool.tile([128, 256], bf16, tag=f"wvT{ct}", name=f"wvT{ct}") for ct in range(2)]
            wnqkT = [cpool.tile([128, 16], bf16, tag=f"wnqkT{ct}", name=f"wnqkT{ct}") for ct in range(2)]
            wnvT = [cpool.tile([128, 256], bf16, tag=f"wnvT{ct}", name=f"wnvT{ct}") for ct in range(2)]
            kw = [cpool.tile([128, 2304], bf16, tag=f"kw{ct}", name=f"kw{ct}") for ct in range(2)]
            acoef = [cpool.tile([128, 1], f32, tag=f"ac{ct}", name=f"ac{ct}") for ct in range(2)]
            bcoef = [cpool.tile([128, 1], f32, tag=f"bc{ct}", name=f"bc{ct}") for ct in range(2)]
            bqk = cpool.tile([1, 64], bf16, tag="bqk")
            bv = cpool.tile([1, 256], bf16, tag="bv")
            bnqk = cpool.tile([1, 16], bf16, tag="bnqk")
            bnv = cpool.tile([1, 256], bf16, tag="bnv")
            ident = cpool.tile([128, 128], bf16, tag="ident")
            diagmask = cpool.tile([128, 512], bf16, tag="diagmask")
            ones = cpool.tile([128, 128], bf16, tag="ones")
            ones_row = cpool.tile([1, 512], bf16, tag="ones_row")
            for ct in range(2):
                nc.sync.dma_start(out=wqkT[ct], in_=wqkT_d[ct])
                nc.sync.dma_start(out=wvT[ct], in_=wvT_d[ct])
                nc.sync.dma_start(out=wnqkT[ct], in_=wnqkT_d[ct])
                nc.sync.dma_start(out=wnvT[ct], in_=wnvT_d[ct])
                nc.sync.dma_start(out=kw[ct], in_=kw_d[ct])
                nc.sync.dma_start(out=acoef[ct], in_=ac_d[ct])
                nc.sync.dma_start(out=bcoef[ct], in_=bc_d[ct])
            nc.sync.dma_start(out=bqk, in_=bqk_d[:, :])
            nc.sync.dma_start(out=bv, in_=bv_d[:, :])
            nc.sync.dma_start(out=bnqk, in_=bnqk_d[:, :])
            nc.sync.dma_start(out=bnv, in_=bnv_d[:, :])
            nc.sync.dma_start(out=ident, in_=id_d[:, :])
            nc.sync.dma_start(out=diagmask, in_=dm_d[:, :])
            nc.vector.memset(ones, 1.0)
            nc.vector.memset(ones_row, 1.0)

            ctx_d = dpool.tile([2, 128, 16384], bf16)

            # ================= GCA gates (pool + tiny nonlocal + sigmoid)
            pooled = [cpool.tile([128, 4], bf16, tag=f"pooled{ct}", name=f"pooled{ct}") for ct in range(2)]
            for p in range(4):
                for ct in range(2):
                    xt = xpool.tile([128, NPIX], bf16, tag="x")
                    nc.sync.dma_start(out=xt, in_=xp_d[2 * p, ct])
                    nc.vector.tensor_reduce(out=pooled[ct][:, p:p + 1], in_=xt,
                                            axis=mybir.AxisListType.X, op=OP.max)
            # qn/kn: [8,4] each
            qn_ps = pspool.tile([8, 4], f32, tag="mm")
            kn_ps = pspool.tile([8, 4], f32, tag="mm")
            for ct in range(2):
                nc.tensor.matmul(qn_ps, wnqkT[ct][:, 0:8], pooled[ct],
                                 start=(ct == 0), stop=False)
            nc.tensor.matmul(qn_ps, bnqk[:, 0:8], ones_row[:, 0:4],
                             start=False, stop=True)
            for ct in range(2):
                nc.tensor.matmul(kn_ps, wnqkT[ct][:, 8:16], pooled[ct],
                                 start=(ct == 0), stop=False)
            nc.tensor.matmul(kn_ps, bnqk[:, 8:16], ones_row[:, 0:4],
                             start=False, stop=True)
            qn_sb = spool.tile([8, 4], bf16, tag="qn")
            kn_sb = spool.tile([8, 4], bf16, tag="kn")
            nc.vector.tensor_copy(qn_sb, qn_ps)
            nc.vector.tensor_copy(kn_sb, kn_ps)
            e4_ps = pspool.tile([4, 4], f32, tag="mm")
            nc.tensor.matmul(e4_ps, qn_sb, kn_sb, start=True, stop=True)
            e4_sb = spool.tile([4, 4], f32, tag="e4")
            nc.scalar.activation(e4_sb, e4_ps, AF.Exp)
            z4 = spool.tile([4, 1], f32, tag="z4")
            nc.vector.tensor_reduce(out=z4, in_=e4_sb, axis=mybir.AxisListType.X,
                                    op=OP.add)
            r4 = spool.tile([4, 1], f32, tag="r4")
            nc.vector.reciprocal(r4, z4)
            att4 = spool.tile([4, 4], bf16, tag="att4")
            nc.vector.tensor_scalar(out=att4, in0=e4_sb, scalar1=r4, scalar2=None,
                                    op0=OP.mult)
            at4_ps = pspool.tile([4, 4], bf16, tag="mm")
            nc.tensor.transpose(at4_ps, att4, ident[0:4, 0:4])
            attT4 = spool.tile([4, 4], bf16, tag="attT4")
            nc.vector.tensor_copy(attT4, at4_ps)
            vnT_ps = pspool.tile([4, 256], f32, tag="mm")
            for ct in range(2):
                nc.tensor.matmul(vnT_ps, pooled[ct], wnvT[ct],
                                 start=(ct == 0), stop=False)
            nc.tensor.matmul(vnT_ps, ones_row[:, 0:4], bnv, start=False, stop=True)
            vnT_sb = spool.tile([4, 256], bf16, tag="vnT")
            nc.vector.tensor_copy(vnT_sb, vnT_ps)
            gca = [cpool.tile([128, 4], f32, tag=f"gca{ct}", name=f"gca{ct}") for ct in range(2)]
            for ct in range(2):
                g_ps = pspool.tile([128, 4], f32, tag="mm")
                nc.tensor.matmul(g_ps, vnT_sb[:, ct * 128:(ct + 1) * 128], attT4,
                                 start=True, stop=True)
                gt = spool.tile([128, 4], f32, tag="gt")
                nc.vector.scalar_tensor_tensor(out=gt, in0=g_ps, scalar=float(NLG),
                                               in1=pooled[ct], op0=OP.mult,
                                               op1=OP.add)
                nc.scalar.activation(gca[ct], gt, AF.Sigmoid)

            # ================= main loop: 8 images (4 patches x 2 branches)
            O_b0 = [opool.tile([128, NPIX], bf16, tag=f"o0_{ct}", name=f"o0_{ct}") for ct in range(2)]
            O_b1 = [opool.tile([128, NPIX], bf16, tag=f"o1_{ct}", name=f"o1_{ct}") for ct in range(2)]
            x_b0 = [None, None]

            for img in range(8):
                p, br = img // 2, img % 2
                O = O_b0 if br == 0 else O_b1

                xt = [xpool.tile([128, NPIX], bf16, tag="x", name="xt") for _ in range(2)]
                for ct in range(2):
                    nc.sync.dma_start(out=xt[ct], in_=xp_d[img, ct])
                if br == 0:
                    x_b0 = xt

                # ---- projections q,k (psum [64, 512] chunks)
                q_sb = qkpool.tile([32, NPIX], bf16, tag="qk")
                k_sb = qkpool.tile([32, NPIX], bf16, tag="qk")
                for chv in range(8):
                    sl = slice(chv * 512, (chv + 1) * 512)
                    ps = pspool.tile([64, 512], f32, tag="mm")
                    for ct in range(2):
                        nc.tensor.matmul(ps, wqkT[ct], xt[ct][:, sl],
                                         start=(ct == 0), stop=False)
                    nc.tensor.matmul(ps, bqk, ones_row, start=False, stop=True)
                    nc.vector.tensor_copy(q_sb[:, sl], ps[0:32, :])
                    nc.vector.tensor_copy(k_sb[:, sl], ps[32:64, :])
                # ---- projection v
                v_sb = [wpool.tile([128, NPIX], bf16, tag=f"v_{ct}", name=f"v_{ct}") for ct in range(2)]
                for co in range(2):
                    for chv in range(8):
                        sl = slice(chv * 512, (chv + 1) * 512)
                        ps = pspool.tile([128, 512], f32, tag="mm")
                        for ci in range(2):
                            nc.tensor.matmul(ps, wvT[ci][:, co * 128:(co + 1) * 128],
                                             xt[ci][:, sl], start=(ci == 0), stop=False)
                        nc.tensor.matmul(ps, bv[:, co * 128:(co + 1) * 128],
                                         ones_row, start=False, stop=True)
                        nc.scalar.activation(v_sb[co][:, sl], ps, AF.Copy)

                # ---- v transposes: vT2 (column-major slices), all base partition 0
                vT2 = [wpool.tile([64, 8192], bf16, tag=f"vt2_{ct}", name=f"vt2_{ct}")
                       for ct in range(2)]
                for ct in range(2):
                    for qb in range(16):  # 4 single-col transposes per psum tile
                        tps = trpool.tile([64, 512], bf16, tag="tr", name="tps")
                        for k4 in range(4):
                            w = qb * 4 + k4
                            in1 = fview(v_sb[ct], w, [[64, 64]])
                            nc.tensor.matmul(tps[:, k4 * 128:(k4 + 1) * 128],
                                             in1, ident, start=True, stop=True,
                                             is_transpose=True)
                        nc.scalar.activation(vT2[ct][:, qb * 512:(qb + 1) * 512],
                                             tps, AF.Copy)

                # ---- eH scores + softmax numerators + ZH (+ oH below)
                attH = wpool.tile([64, 4096], bf16, tag="attH")
                attW = wpool.tile([64, 4096], bf16, tag="attW")
                Zt = wpool.tile([128, NPIX], bf16, tag="Zt")
                for grp in range(8):
                    eps = pspool.tile([64, 512], f32, tag="mm", name="eps")
                    for wl in range(8):
                        w = grp * 8 + wl
                        lhsT = fview(k_sb, w, [[64, 64]])
                        rhs = fview(q_sb, w, [[64, 64]])
                        nc.tensor.matmul(eps[:, wl * 64:(wl + 1) * 64], lhsT, rhs,
                                         start=True, stop=True)
                    asl = attH[:, grp * 512:(grp + 1) * 512]
                    nc.scalar.activation(asl, eps, AF.Exp)
                    nc.vector.tensor_mul(asl, asl, diagmask[0:64, :])
                    zH = pspool.tile([128, 512], f32, tag="mm", name="zH")
                    nc.tensor.matmul(zH, ones[0:64, :], asl, start=True, stop=True)
                    nc.vector.tensor_copy(
                        fview(Zt, grp * 8, [[1, 8], [64, 64]]),
                        fview(zH, 0, [[64, 8], [1, 64]]))
                # ---- oH (uses vT2)
                for grp8 in range(8):
                    for ct in range(2):
                        ops_ = pspool.tile([128, 512], f32, tag="mm", name="ops")
                        for wl in range(8):
                            w = grp8 * 8 + wl
                            lhsT = fview(vT2[ct], w * 128, [[1, 128]])
                            rhs = fview(attH, w * 64, [[1, 64]])
                            nc.tensor.matmul(ops_[:, wl * 64:(wl + 1) * 64],
                                             lhsT, rhs, start=True, stop=True)
                        nc.vector.tensor_copy(
                            fview(O[ct], grp8 * 8, [[1, 8], [64, 64]]),
                            fview(ops_, 0, [[64, 8], [1, 64]]))

                # ---- vT3 (row slices) into the SAME buffers as vT2 (oH done)
                vT3 = [wpool.tile([64, 8192], bf16, tag=f"vt2_{ct}", name=f"vt3_{ct}")
                       for ct in range(2)]
                for ct in range(2):
                    for qb in range(16):
                        tps = trpool.tile([64, 512], bf16, tag="tr", name="tps3")
                        for k4 in range(4):
                            h = qb * 4 + k4
                            in1 = fview(v_sb[ct], h * 64, [[1, 64]])
                            nc.tensor.matmul(tps[:, k4 * 128:(k4 + 1) * 128],
                                             in1, ident, start=True, stop=True,
                                             is_transpose=True)
                        nc.scalar.activation(vT3[ct][:, qb * 512:(qb + 1) * 512],
                                             tps, AF.Copy)
                # ---- eW + ZW + oW
                for grp in range(8):
                    fps = pspool.tile([64, 512], f32, tag="mm", name="fps")
                    for hl in range(8):
                        h = grp * 8 + hl
                        lhsT = fview(k_sb, h * 64, [[1, 64]])
                        rhs = fview(q_sb, h * 64, [[1, 64]])
                        nc.tensor.matmul(fps[:, hl * 64:(hl + 1) * 64], lhsT, rhs,
                                         start=True, stop=True)
                    asl = attW[:, grp * 512:(grp + 1) * 512]
                    nc.scalar.activation(asl, fps, AF.Exp)
                    zW = pspool.tile([128, 512], f32, tag="mm", name="zW")
                    nc.tensor.matmul(zW, ones[0:64, :], asl, start=True, stop=True)
                    sl_e = fview(Zt, (grp * 8) * 64, [[64, 8], [1, 64]])
                    nc.vector.tensor_tensor(out=sl_e, in0=sl_e,
                                            in1=fview(zW, 0, [[64, 8], [1, 64]]),
                                            op=OP.add)
                for grp8 in range(8):
                    for ct in range(2):
                        ops_ = pspool.tile([128, 512], f32, tag="mm", name="opsw")
                        for hl in range(8):
                            h = grp8 * 8 + hl
                            lhsT = fview(vT3[ct], h * 128, [[1, 128]])
                            rhs = fview(attW, h * 64, [[1, 64]])
                            nc.tensor.matmul(ops_[:, hl * 64:(hl + 1) * 64],
                                             lhsT, rhs, start=True, stop=True)
                        osl = fview(O[ct], grp8 * 8 * 64, [[64, 8], [1, 64]])
                        nc.vector.tensor_tensor(
                            out=osl, in0=osl,
                            in1=fview(ops_, 0, [[64, 8], [1, 64]]), op=OP.add)

                # ---- O *= gamma * R  (R = 1/Zt), chunked f32 reciprocal
                gsc = float(G1) if br == 0 else float(G2)
                for chv in range(4):
                    sl = slice(chv * 1024, (chv + 1) * 1024)
                    rch = spool.tile([128, 1024], f32, tag="rch")
                    nc.vector.reciprocal(rch, Zt[:, sl])
                    for ct in range(2):
                        nc.vector.scalar_tensor_tensor(
                            out=O[ct][:, sl], in0=O[ct][:, sl], scalar=gsc,
                            in1=rch, op0=OP.mult, op1=OP.mult)

                if br == 1:
                    # ---- rotate O_b1 back by +45 into rotG; combine; write ctx
                    for ct in range(2):
                        tmpA = wpool.tile([128, 8192], bf16, tag="vt2_0")
                        nc.gpsimd.memset(tmpA, 0.0)
                        for (v0, c0, L) in runsA:
                            nc.vector.tensor_copy(
                                fview(tmpA, v0, [[128, 64], [1, L]]),
                                fview(O_b1[ct], c0, [[64, 64], [1, L]]))
                        rotG = wpool.tile([128, NPIX], bf16, tag="v_1")
                        nc.gpsimd.memset(rotG, 0.0)
                        for u in range(-63, 64):
                            if u not in Ru:
                                continue
                            y0 = max(0, u)
                            y1 = min(63, 63 + u)
                            L = y1 - y0 + 1
                            nc.vector.tensor_copy(
                                fview(rotG, 65 * y0 - u, [[65, L]]),
                                fview(tmpA, Ru[u] * 128 + 2 * y0 - u, [[2, L]]))
                        # ctx = (O_b0 + rotG + x) * gate
                        nc.vector.tensor_tensor(out=O_b0[ct], in0=O_b0[ct],
                                                in1=rotG, op=OP.add)
                        nc.vector.tensor_tensor(out=O_b0[ct], in0=O_b0[ct],
                                                in1=x_b0[ct], op=OP.add)
                        ctxt = wpool.tile([128, NPIX], bf16, tag="v_0")
                        nc.vector.tensor_scalar(out=ctxt, in0=O_b0[ct],
                                                scalar1=gca[ct][:, p:p + 1],
                                                scalar2=None, op0=OP.mult)
                        pr, pc = p // 2, p % 2
                        dst = dview(ctx_d, ct * (128 * 16384) + (pr * 64) * 128
                                    + pc * 64,
                                    [[16384, 128], [128, 64], [1, 64]])
                        nc.sync.dma_start(out=dst, in_=ctxt)

            # ================= conv 3x3 + BN + residual ReLU
            for chk in range(32):
                r0 = chk * 4
                vs = max(0, r0 - 1)
                ve = min(127, r0 + 4)
                nrows = ve - vs + 1
                j0 = vs - (r0 - 1)
                cc = []
                for ct in range(2):
                    t = ccpool.tile([128, 780], bf16, tag=f"cc{ct}")
                    nc.gpsimd.memset(t, 0.0)
                    src = dview(ctx_d, ct * (128 * 16384) + vs * 128,
                                [[16384, 128], [128, nrows], [1, 128]])
                    nc.sync.dma_start(
                        out=fview(t, j0 * 130 + 1, [[130, nrows], [1, 128]]),
                        in_=src)
                    cc.append(t)
                for ko in range(2):
                    ps = pspool.tile([128, 512], f32, tag="mm")
                    first = True
                    for tap in range(9):
                        dy, dx = tap // 3, tap % 3
                        for ki in range(2):
                            last = (tap == 8 and ki == 1)
                            nc.tensor.matmul(
                                ps, kw[ki][:, (tap * 2 + ko) * 128:
                                           (tap * 2 + ko + 1) * 128],
                                fview(cc[ki], dy * 130 + dx, [[130, 4], [1, 128]]),
                                start=first, stop=last)
                            first = False
                    xrt = copool.tile([128, 512], bf16, tag="xr")
                    nc.sync.dma_start(out=xrt,
                                      in_=xres_d[ko, :, r0 * 128:(r0 + 4) * 128])
                    t1 = copool.tile([128, 512], bf16, tag="t1")
                    nc.vector.tensor_scalar(out=t1, in0=ps, scalar1=acoef[ko],
                                            scalar2=bcoef[ko], op0=OP.mult,
                                            op1=OP.add)
                    t2 = copool.tile([128, 512], bf16, tag="t2")
                    nc.vector.tensor_tensor(out=t2, in0=t1, in1=xrt, op=OP.add)
                    osb = copool.tile([128, 512], bf16, tag="osb")
                    nc.scalar.activation(osb, t2, AF.Relu)
                    nc.sync.dma_start(out=out_d[ko, :, r0 * 128:(r0 + 4) * 128],
                                      in_=osb)

    nc.finalize()
    return nc


def _get_program(g1, g2, nlg):
    key = (round(float(g1), 10), round(float(g2), 10), round(float(nlg), 10))
    if key not in _PROG_CACHE:
        _PROG_CACHE[key] = _build_program(g1, g2, nlg)
    return _PROG_CACHE[key]


# ---------------------------------------------------------------- host glue
def _prep_inputs(x, wq, bq, wk, bk, wv, bv, nq_w, nq_b, nk_w, nk_b, nv_w, nv_b,
                 conv_w, conv_b, bn_w, bn_b, bn_mean, bn_var, gamma):
    import ml_dtypes
    bf = ml_dtypes.bfloat16
    B = x.shape[0]
    f32 = np.float32

    iy, ix, valid = _rotate_map(HS, HS, -45.0)
    gmap = (iy * HS + ix).reshape(-1)
    vmask = valid.reshape(-1).astype(f32)

    # patches: xp[img, ct, c_local, pix]; img = 2*p + b, p = pr*2+pc
    x5 = x.reshape(B, 256, 2, HS, 2, HS).transpose(0, 2, 4, 1, 3, 5)
    # x5: (B, pr, pc, C, h, w)
    xp_all = []
    for n in range(B):
        per = np.empty((8, 2, 128, NPIX), dtype=bf)
        for p in range(4):
            pr, pc = p // 2, p % 2
            img = x5[n, pr, pc].reshape(256, NPIX).astype(f32)
            imgr = img[:, gmap] * vmask[None, :]
            per[2 * p + 0] = img.reshape(2, 128, NPIX).astype(bf)
            per[2 * p + 1] = imgr.reshape(2, 128, NPIX).astype(bf)
        xp_all.append(per)

    xres_all = [x[n].reshape(2, 128, 16384).astype(bf) for n in range(B)]

    wqkT = np.stack([np.concatenate([wq.T, wk.T], axis=1)[ct * 128:(ct + 1) * 128]
                     for ct in range(2)]).astype(bf)  # (2,128,64)
    bqk = np.concatenate([bq, bk])[None, :].astype(bf)
    wvT = np.stack([wv.T[ct * 128:(ct + 1) * 128] for ct in range(2)]).astype(bf)
    bvr = bv[None, :].astype(bf)
    wnqkT = np.stack([np.concatenate([nq_w.T, nk_w.T], axis=1)
                      [ct * 128:(ct + 1) * 128] for ct in range(2)]).astype(bf)
    bnqk = np.concatenate([nq_b, nk_b])[None, :].astype(bf)
    wnvT = np.stack([nv_w.T[ct * 128:(ct + 1) * 128] for ct in range(2)]).astype(bf)
    bnv = nv_b[None, :].astype(bf)

    kw = np.empty((2, 128, 2304), dtype=bf)
    for ki in range(2):
        for tap in range(9):
            dy, dx = tap // 3, tap % 3
            for ko in range(2):
                blk = conv_w[ko * 128:(ko + 1) * 128,
                             ki * 128:(ki + 1) * 128, dy, dx].T  # (128 in,128 out)
                kw[ki, :, (tap * 2 + ko) * 128:(tap * 2 + ko + 1) * 128] = \
                    blk.astype(bf)

    rs = 1.0 / np.sqrt(bn_var + 1e-5)
    A = (gamma * bn_w * rs).astype(f32)
    Bc = (gamma * ((conv_b - bn_mean) * rs * bn_w + bn_b)).astype(f32)
    ac = A.reshape(2, 128, 1)
    bc = Bc.reshape(2, 128, 1)

    ident = np.eye(128, dtype=f32).astype(bf)
    dm = np.ones((128, 512), dtype=f32)
    for pp in range(128):
        g = pp % 64
        for blk in range(8):
            dm[pp, blk * 64 + g] = 0.0
    dm = dm.astype(bf)

    consts = dict(wqkT=wqkT, bqk=bqk, wvT=wvT, bv=bvr, wnqkT=wnqkT, bnqk=bnqk,
                  wnvT=wnvT, bnv=bnv, kw=kw, acoef=ac, bcoef=bc, ident=ident,
                  diagmask=dm, scal=np.zeros((1, 8), np.float32))
    return xp_all, xres_all, consts


def kernel(**inputs) -> np.ndarray:
    x = np.asarray(inputs["x"], np.float32)
    assert int(inputs["scale"]) == 2
    g1 = float(np.asarray(inputs["gamma1"]))
    g2 = float(np.asarray(inputs["gamma2"]))
    nlg = float(np.asarray(inputs["nl_gamma"]))
    gamma = float(np.asarray(inputs["gamma"]))

    xp_all, xres_all, consts = _prep_inputs(
        x, np.asarray(inputs["wq"], np.float32), np.asarray(inputs["bq"], np.float32),
        np.asarray(inputs["wk"], np.float32), np.asarray(inputs["bk"], np.float32),
        np.asarray(inputs["wv"], np.float32), np.asarray(inputs["bv"], np.float32),
        np.asarray(inputs["nq_w"], np.float32), np.asarray(inputs["nq_b"], np.float32),
        np.asarray(inputs["nk_w"], np.float32), np.asarray(inputs["nk_b"], np.float32),
        np.asarray(inputs["nv_w"], np.float32), np.asarray(inputs["nv_b"], np.float32),
        np.asarray(inputs["conv_w"], np.float32), np.asarray(inputs["conv_b"], np.float32),
        np.asarray(inputs["bn_w"], np.float32), np.asarray(inputs["bn_b"], np.float32),
        np.asarray(inputs["bn_mean"], np.float32), np.asarray(inputs["bn_var"], np.float32),
        gamma)

    nc = _get_program(g1, g2, nlg)

    in_maps = []
    for c in range(8):
        m = dict(consts)
        m["xp"] = xp_all[c]
        m["xres"] = xres_all[c]
        in_maps.append(m)

    if os.environ.get("AGCB_SIM", "0") == "1":
        from concourse.bass_interp import CoreSim
        sim = CoreSim(nc)
        for name, arr in in_maps[0].items():
            sim.tensor(name)[:] = arr
        sim.simulate()
        outs = [np.asarray(sim.tensor("out")).astype(np.float32)]
        res = np.repeat(outs[0][None], 8, axis=0)
    else:
        from concourse.bass_utils import run_bass_kernel_spmd
        trace = os.environ.get("AGCB_TRACE", "0") == "1"
        r = run_bass_kernel_spmd(nc, in_maps, list(range(8)), trace=trace)
        global _LAST_EXEC_NS
        _LAST_EXEC_NS = r.exec_time_ns
        if trace and r.instructions_and_trace is not None:
            print(f"[kernel] trace: {r.instructions_and_trace[1]}", file=sys.stderr)
        res = np.stack([np.asarray(rm["out"]).astype(np.float32) for rm in r.results])

    out = res.reshape(8, 256, 128, 128).astype(np.float32)
    return out


_LAST_EXEC_NS = None
